# revision 26
# baseline (speedup 1.0000x reference)
"""Trainium2 kernel for DWTFeatureModel.

Model: 3-level db4 DWT along time (256 -> 276 coeffs, reflect padding) for
each of B*64 channels, then a Conv3d whose kernel spans the whole
(276, 8, 8) volume (== full contraction to 64 features), bias, LeakyReLU.

The DWT is linear, so dwt(sig) = sig @ M for a fixed (256, 276) analysis
matrix M built from the db4 filter bank. The whole model then collapses to

    out[b, f] = leaky(sum_{s,hw} x[b, s, hw] * Weff[s, hw, f] + bias[f])
    Weff[s, hw, f] = sum_t M[s, t] * W[f, t, hw]

Device kernel (per core, pure batch-data-parallel over 8 cores):
  warmup:          dummy matmuls on a zeroed tile keep the PE busy while
                   the weight DMAs land, so the HAM clock gate reaches
                   2.4 GHz before real work starts.
  phase 1 (fold):  Weff = M^T-contraction of the replicated conv weight,
                   48 N=512 bf16 matmuls, laid out directly as the
                   stationary operands of phase 2.
  phase 2 (main):  out^T = Weff^T @ x^T as 128 accumulating matmuls
                   (K=128, N=256 batch columns) in bf16 with fp32 PSUM,
                   streaming x (8 MB/core) as 4 contiguous 2 MB tiles.
                   Contraction chunks are ordered sblk-major to match the
                   fold's output order, so phase 2 starts as soon as the
                   first fold block is evacuated.
  epilogue:        + bias, LeakyReLU via max(y, 0.02*y), DMA out.

Host side only shards/permutes/casts inputs (x^T tiles per core) and
transposes the (64, 256) per-core outputs back.
"""

from contextlib import ExitStack

import numpy as np

import concourse.bass as bass
import concourse.tile as tile
from concourse import bacc, mybir
from concourse.bass_utils import run_bass_kernel_spmd

# pywt db4 analysis filters (identical constants to the model definition)
DEC_LO = [-0.010597401784997278, 0.032883011666982945, 0.030841381835986965,
          -0.18703481171888114, -0.02798376941698385, 0.6308807679295904,
          0.7148465705525415, 0.23037781330885523]
DEC_HI = [-0.23037781330885523, 0.7148465705525415, -0.6308807679295904,
          -0.02798376941698385, 0.18703481171888114, 0.030841381835986965,
          -0.032883011666982945, -0.010597401784997278]

B, T, F, TDWT = 2048, 256, 64, 276
J, L = 3, 8
NEG_SLOPE = 0.02
NCORES = 8
BC = B // NCORES          # 256 batches per core
G = 128                   # contraction chunks of 128 (= 2 s-blocks x 64 hw)
# x tile sizes in chunks: big tiles first, tapered at the end so the PE
# isn't left with a long chase after the last DMA byte lands
XTILES = [16, 16, 16, 16, 16, 16, 16, 8, 8]
NT = len(XTILES)
NWARM = 8                 # PE warmup matmuls
TCH = [(0, 128), (128, 128), (256, 20)]  # t-chunks of the 276 DWT coeffs


def _build_dwt_matrix():
    """M (T, TDWT) with dwt(sig) = sig @ M, matching the reference's
    multi-level reflect-padded strided cross-correlation."""
    h_lo = np.array(DEC_LO, np.float64)[::-1]
    h_hi = np.array(DEC_HI, np.float64)[::-1]
    lo = np.eye(T, dtype=np.float64)
    his = []
    for _ in range(J):
        n = lo.shape[-1]
        outsize = (n + L - 1) // 2
        p = 2 * (outsize - 1) - n + L
        xp = np.pad(lo, ((0, 0), (p // 2, (p + 1) // 2)), mode="reflect")
        idx = np.arange(outsize)[:, None] * 2 + np.arange(L)[None, :]
        win = xp[:, idx]
        his.append(win @ h_hi)
        lo = win @ h_lo
    return np.concatenate([lo] + his, axis=-1)  # (256, 276)


def _emit(ctx, tc, xt, wt, bi, outT):
    nc = tc.nc
    f32 = mybir.dt.float32
    bf16 = mybir.dt.bfloat16

    const_pool = ctx.enter_context(tc.tile_pool(name="const", bufs=1))
    weff_pool = ctx.enter_context(tc.tile_pool(name="weff", bufs=1))
    xpool = ctx.enter_context(tc.tile_pool(name="x", bufs=1))
    fold_ps = ctx.enter_context(tc.tile_pool(name="foldps", bufs=7, space="PSUM"))
    out_ps = ctx.enter_context(tc.tile_pool(name="outps", bufs=1, space="PSUM"))
    opool = ctx.enter_context(tc.tile_pool(name="osb", bufs=1))

    # ---- PE warmup: zero matmuls with no DMA dependency, into the acc
    # bank (the first real matmul's start=True clears it)
    acc = out_ps.tile([F, BC], f32)
    wsrc = const_pool.tile([128, BC], bf16, tag="warm")
    nc.gpsimd.memset(wsrc[:], 0.0)
    for _ in range(NWARM):
        nc.tensor.matmul(acc[0:1, :], wsrc[:, 0:1], wsrc[:], start=True, stop=True)

    # ---- constants (single FIFO ring: dm+wt combined blob first, then x,
    # then bias, which is only needed by the epilogue)
    cw_sb = []
    for ti, (t0, tsz) in enumerate(TCH):
        cw = const_pool.tile([tsz, T + 64 * F], bf16, tag=f"cw{ti}")
        nc.sync.dma_start(cw[:], wt[t0:t0 + tsz, :])
        cw_sb.append(cw)
    dm_sb = [cw[:, 0:T] for cw in cw_sb]
    wt_sb = [cw[:, T:] for cw in cw_sb]

    xt_tiles = []
    off = 0
    for t, xg in enumerate(XTILES):
        xt_tile = xpool.tile([128, xg, BC], bf16, tag=f"xt{t}")
        foff = 128 * BC * off
        src = xt[foff: foff + 128 * xg * BC].rearrange(
            "(p c b) -> p c b", p=128, c=xg)
        nc.sync.dma_start(xt_tile[:], src)
        xt_tiles.append(xt_tile)
        off += xg

    bias = const_pool.tile([F, 1], f32, tag="bias")
    nc.sync.dma_start(bias[:], bi[:])

    # ---- fold: weff[s_in, sblk*4096 + hw*64 + f] = sum_t D[t,s] W[f,t,hw]
    # One N=512 matmul covers 8 hw x 64 f contiguous output columns.
    # t-chunk-outer within waves of 4 groups, so the first matmuls only
    # depend on the first wt chunk's DMA.
    weff = weff_pool.tile([128, 2 * 64 * F], bf16)
    for wave in range(4):
        pws = [fold_ps.tile([128, 512], f32, tag="pw", name=f"pw_{wave}_{k}")
               for k in range(4)]
        for ti, (t0, tsz) in enumerate(TCH):
            for k in range(4):
                grp = wave * 4 + k
                sblk, hwg = grp // 8, grp % 8
                nc.tensor.matmul(
                    pws[k][:],
                    dm_sb[ti][:, sblk * 128:(sblk + 1) * 128],
                    wt_sb[ti][:, hwg * 512:(hwg + 1) * 512],
                    start=(ti == 0), stop=(ti == 2),
                )
        for k in range(4):
            grp = wave * 4 + k
            sblk, hwg = grp // 8, grp % 8
            dst = weff[:, sblk * 4096 + hwg * 512: sblk * 4096 + (hwg + 1) * 512]
            if k % 2 == 0:
                nc.vector.tensor_copy(dst, pws[k][:])
            else:
                nc.scalar.copy(dst, pws[k][:])

    # ---- main: out^T[f, b] accumulated over 128 sblk-major chunks
    off = 0
    for t, xg in enumerate(XTILES):
        for c in range(xg):
            g = off + c
            sblk, hw = g // 64, g % 64
            nc.tensor.matmul(
                acc[:],
                weff[:, sblk * 4096 + hw * 64: sblk * 4096 + (hw + 1) * 64],
                xt_tiles[t][:, c, :],
                start=(g == 0), stop=(g == G - 1),
            )
        off += xg

    # ---- epilogue: bias + LeakyReLU, store
    t1 = opool.tile([F, BC], f32)
    y = opool.tile([F, BC], f32)
    nc.vector.tensor_scalar_add(t1[:], acc[:], bias[:])
    nc.vector.scalar_tensor_tensor(
        y[:], t1[:], NEG_SLOPE, t1[:],
        op0=mybir.AluOpType.mult, op1=mybir.AluOpType.max,
    )
    nc.sync.dma_start(outT[:], y[:])


_CACHE = {}


def _get_kernel():
    if "nc" not in _CACHE:
        nc = bacc.Bacc("TRN2", target_bir_lowering=False, debug=False)
        f32 = mybir.dt.float32
        bf16 = mybir.dt.bfloat16
        xt_d = nc.dram_tensor("xt", [G * 128 * BC], bf16, kind="ExternalInput")
        wt_d = nc.dram_tensor("wt", [TDWT, T + 64 * F], bf16, kind="ExternalInput")
        bi_d = nc.dram_tensor("bi", [F, 1], f32, kind="ExternalInput")
        out_d = nc.dram_tensor("outT", [F, BC], f32, kind="ExternalOutput")
        with tile.TileContext(nc) as tc, ExitStack() as ctx:
            _emit(ctx, tc, xt_d.ap(), wt_d.ap(), bi_d.ap(), out_d.ap())
        nc.compile()
        _CACHE["nc"] = nc
    return _CACHE["nc"]


def make_in_maps(x, W, b):
    import ml_dtypes
    bf16 = ml_dtypes.bfloat16
    dwt_m = _build_dwt_matrix()
    dm = dwt_m.T                                                   # (276, 256)
    wtc = W[:, 0].reshape(F, TDWT, 64).transpose(1, 2, 0).reshape(TDWT, 64 * F)
    wt = np.ascontiguousarray(
        np.concatenate([dm, wtc], axis=1)).astype(bf16)            # (276, 256+4096)
    bi = np.ascontiguousarray(b.reshape(F, 1)).astype(np.float32)
    in_maps = []
    for c in range(NCORES):
        # chunk g = sblk*64 + hw holds rows [s_in, b]; tiles of XTILES[t]
        # chunks are stored back-to-back as [p, chunk, b] blocks so each
        # tile is one contiguous DMA.
        xc = x[c * BC:(c + 1) * BC, 0].astype(bf16)                # (BC, 256, 8, 8)
        xg = xc.reshape(BC, 2, 128, 64).transpose(1, 3, 2, 0)      # (sblk, hw, s_in, b)
        xg = xg.reshape(G, 128, BC)                                # (g, p, b)
        parts, off = [], 0
        for n in XTILES:
            parts.append(np.ascontiguousarray(
                xg[off:off + n].transpose(1, 0, 2)).reshape(-1))   # (p, c, b) flat
            off += n
        in_maps.append({
            "xt": np.concatenate(parts), "wt": wt, "bi": bi,
        })
    return in_maps


def kernel(x, W, b, _trace=False):
    nc = _get_kernel()
    in_maps = make_in_maps(np.asarray(x), np.asarray(W), np.asarray(b))
    res = run_bass_kernel_spmd(nc, in_maps, list(range(NCORES)), trace=_trace)
    out = np.empty((B, F), np.float32)
    for c in range(NCORES):
        out[c * BC:(c + 1) * BC] = res.results[c]["outT"].T
    if _trace:
        return out, res
    return out


# revision 28
# speedup vs baseline: 1.0161x; 1.0161x over previous
"""Trainium2 kernel for DWTFeatureModel.

Model: 3-level db4 DWT along time (256 -> 276 coeffs, reflect padding) for
each of B*64 channels, then a Conv3d whose kernel spans the whole
(276, 8, 8) volume (== full contraction to 64 features), bias, LeakyReLU.

The DWT is linear, so dwt(sig) = sig @ M for a fixed (256, 276) analysis
matrix M built from the db4 filter bank. The whole model then collapses to

    out[b, f] = leaky(sum_{s,hw} x[b, s, hw] * Weff[s, hw, f] + bias[f])
    Weff[s, hw, f] = sum_t M[s, t] * W[f, t, hw]

Device kernel (per core, pure batch-data-parallel over 8 cores):
  warmup:          dummy matmuls on a zeroed tile keep the PE busy while
                   the weight DMAs land, so the HAM clock gate reaches
                   2.4 GHz before real work starts.
  phase 1 (fold):  Weff = M^T-contraction of the replicated conv weight,
                   48 N=512 bf16 matmuls, laid out directly as the
                   stationary operands of phase 2.
  phase 2 (main):  out^T = Weff^T @ x^T as 128 accumulating matmuls
                   (K=128, N=256 batch columns) in bf16 with fp32 PSUM,
                   streaming x (8 MB/core) as 4 contiguous 2 MB tiles.
                   Contraction chunks are ordered sblk-major to match the
                   fold's output order, so phase 2 starts as soon as the
                   first fold block is evacuated.
  epilogue:        + bias, LeakyReLU via max(y, 0.02*y), DMA out.

Host side only shards/permutes/casts inputs (x^T tiles per core) and
transposes the (64, 256) per-core outputs back.
"""

from contextlib import ExitStack

import numpy as np

import concourse.bass as bass
import concourse.tile as tile
from concourse import bacc, mybir
from concourse.bass_utils import run_bass_kernel_spmd

# pywt db4 analysis filters (identical constants to the model definition)
DEC_LO = [-0.010597401784997278, 0.032883011666982945, 0.030841381835986965,
          -0.18703481171888114, -0.02798376941698385, 0.6308807679295904,
          0.7148465705525415, 0.23037781330885523]
DEC_HI = [-0.23037781330885523, 0.7148465705525415, -0.6308807679295904,
          -0.02798376941698385, 0.18703481171888114, 0.030841381835986965,
          -0.032883011666982945, -0.010597401784997278]

B, T, F, TDWT = 2048, 256, 64, 276
J, L = 3, 8
NEG_SLOPE = 0.02
NCORES = 8
BC = B // NCORES          # 256 batches per core
G = 128                   # contraction chunks of 128 (= 2 s-blocks x 64 hw)
# x tile sizes in chunks: big tiles first, tapered at the end so the PE
# isn't left with a long chase after the last DMA byte lands
XTILES = [16, 16, 16, 16, 16, 16, 16, 8, 8]
NT = len(XTILES)
NWARM = 8                 # PE warmup matmuls
TCH = [(0, 128), (128, 128), (256, 20)]  # t-chunks of the 276 DWT coeffs


def _build_dwt_matrix():
    """M (T, TDWT) with dwt(sig) = sig @ M, matching the reference's
    multi-level reflect-padded strided cross-correlation."""
    h_lo = np.array(DEC_LO, np.float64)[::-1]
    h_hi = np.array(DEC_HI, np.float64)[::-1]
    lo = np.eye(T, dtype=np.float64)
    his = []
    for _ in range(J):
        n = lo.shape[-1]
        outsize = (n + L - 1) // 2
        p = 2 * (outsize - 1) - n + L
        xp = np.pad(lo, ((0, 0), (p // 2, (p + 1) // 2)), mode="reflect")
        idx = np.arange(outsize)[:, None] * 2 + np.arange(L)[None, :]
        win = xp[:, idx]
        his.append(win @ h_hi)
        lo = win @ h_lo
    return np.concatenate([lo] + his, axis=-1)  # (256, 276)


def _emit(ctx, tc, xt, wt, bi, outT):
    nc = tc.nc
    f32 = mybir.dt.float32
    bf16 = mybir.dt.bfloat16

    const_pool = ctx.enter_context(tc.tile_pool(name="const", bufs=1))
    weff_pool = ctx.enter_context(tc.tile_pool(name="weff", bufs=1))
    xpool = ctx.enter_context(tc.tile_pool(name="x", bufs=1))
    fold_ps = ctx.enter_context(tc.tile_pool(name="foldps", bufs=7, space="PSUM"))
    out_ps = ctx.enter_context(tc.tile_pool(name="outps", bufs=1, space="PSUM"))
    opool = ctx.enter_context(tc.tile_pool(name="osb", bufs=1))

    # ---- PE warmup: zero matmuls with no DMA dependency, into the acc
    # bank (the first real matmul's start=True clears it)
    acc = out_ps.tile([F, BC], f32)
    wsrc = const_pool.tile([128, BC], bf16, tag="warm")
    nc.gpsimd.memset(wsrc[:], 0.0)
    for _ in range(NWARM):
        nc.tensor.matmul(acc[0:1, :], wsrc[:, 0:1], wsrc[:], start=True, stop=True)

    # ---- constants (single FIFO ring: dm+wt combined blob first, then x,
    # then bias, which is only needed by the epilogue). Each t-chunk's blob
    # is DMA'd in two column halves so the fold's first waves only gate on
    # the first ~0.6 MB.
    CSPLIT = T + 64 * F // 2  # dm + hwg0-3 columns
    cw_sb = []
    for ti, (t0, tsz) in enumerate(TCH):
        cw = const_pool.tile([tsz, T + 64 * F], bf16, tag=f"cw{ti}")
        nc.sync.dma_start(cw[:, 0:CSPLIT], wt[t0:t0 + tsz, 0:CSPLIT])
        cw_sb.append(cw)
    for ti, (t0, tsz) in enumerate(TCH):
        nc.sync.dma_start(cw_sb[ti][:, CSPLIT:], wt[t0:t0 + tsz, CSPLIT:])
    dm_sb = [cw[:, 0:T] for cw in cw_sb]
    wt_sb = [cw[:, T:] for cw in cw_sb]

    xt_tiles = []
    off = 0
    for t, xg in enumerate(XTILES):
        xt_tile = xpool.tile([128, xg, BC], bf16, tag=f"xt{t}")
        foff = 128 * BC * off
        src = xt[foff: foff + 128 * xg * BC].rearrange(
            "(p c b) -> p c b", p=128, c=xg)
        nc.sync.dma_start(xt_tile[:], src)
        xt_tiles.append(xt_tile)
        off += xg

    bias = const_pool.tile([F, 1], f32, tag="bias")
    nc.sync.dma_start(bias[:], bi[:])

    # ---- fold: weff[s_in, sblk*4096 + hw*64 + f] = sum_t D[t,s] W[f,t,hw]
    # One N=512 matmul covers 8 hw x 64 f contiguous output columns.
    # t-chunk-outer within waves of 4 groups, so the first matmuls only
    # depend on the first wt chunk's DMA.
    weff = weff_pool.tile([128, 2 * 64 * F], bf16)
    WAVES = [[0, 1, 2, 3], [8, 9, 10, 11], [4, 5, 6, 7], [12, 13, 14, 15]]
    for wave, grps in enumerate(WAVES):
        pws = [fold_ps.tile([128, 512], f32, tag="pw", name=f"pw_{wave}_{k}")
               for k in range(4)]
        for ti, (t0, tsz) in enumerate(TCH):
            for k, grp in enumerate(grps):
                sblk, hwg = grp // 8, grp % 8
                nc.tensor.matmul(
                    pws[k][:],
                    dm_sb[ti][:, sblk * 128:(sblk + 1) * 128],
                    wt_sb[ti][:, hwg * 512:(hwg + 1) * 512],
                    start=(ti == 0), stop=(ti == 2),
                )
        for k, grp in enumerate(grps):
            sblk, hwg = grp // 8, grp % 8
            dst = weff[:, sblk * 4096 + hwg * 512: sblk * 4096 + (hwg + 1) * 512]
            if k % 2 == 0:
                nc.vector.tensor_copy(dst, pws[k][:])
            else:
                nc.scalar.copy(dst, pws[k][:])

    # ---- main: out^T[f, b] accumulated over 128 sblk-major chunks
    off = 0
    for t, xg in enumerate(XTILES):
        for c in range(xg):
            g = off + c
            sblk, hw = g // 64, g % 64
            nc.tensor.matmul(
                acc[:],
                weff[:, sblk * 4096 + hw * 64: sblk * 4096 + (hw + 1) * 64],
                xt_tiles[t][:, c, :],
                start=(g == 0), stop=(g == G - 1),
            )
        off += xg

    # ---- epilogue: bias + LeakyReLU, store
    t1 = opool.tile([F, BC], f32)
    y = opool.tile([F, BC], f32)
    nc.vector.tensor_scalar_add(t1[:], acc[:], bias[:])
    nc.vector.scalar_tensor_tensor(
        y[:], t1[:], NEG_SLOPE, t1[:],
        op0=mybir.AluOpType.mult, op1=mybir.AluOpType.max,
    )
    nc.sync.dma_start(outT[:], y[:])


_CACHE = {}


def _get_kernel():
    if "nc" not in _CACHE:
        nc = bacc.Bacc("TRN2", target_bir_lowering=False, debug=False)
        f32 = mybir.dt.float32
        bf16 = mybir.dt.bfloat16
        xt_d = nc.dram_tensor("xt", [G * 128 * BC], bf16, kind="ExternalInput")
        wt_d = nc.dram_tensor("wt", [TDWT, T + 64 * F], bf16, kind="ExternalInput")
        bi_d = nc.dram_tensor("bi", [F, 1], f32, kind="ExternalInput")
        out_d = nc.dram_tensor("outT", [F, BC], f32, kind="ExternalOutput")
        with tile.TileContext(nc) as tc, ExitStack() as ctx:
            _emit(ctx, tc, xt_d.ap(), wt_d.ap(), bi_d.ap(), out_d.ap())
        nc.compile()
        _CACHE["nc"] = nc
    return _CACHE["nc"]


def make_in_maps(x, W, b):
    import ml_dtypes
    bf16 = ml_dtypes.bfloat16
    dwt_m = _build_dwt_matrix()
    dm = dwt_m.T                                                   # (276, 256)
    wtc = W[:, 0].reshape(F, TDWT, 64).transpose(1, 2, 0).reshape(TDWT, 64 * F)
    wt = np.ascontiguousarray(
        np.concatenate([dm, wtc], axis=1)).astype(bf16)            # (276, 256+4096)
    bi = np.ascontiguousarray(b.reshape(F, 1)).astype(np.float32)
    in_maps = []
    for c in range(NCORES):
        # chunk g = sblk*64 + hw holds rows [s_in, b]; tiles of XTILES[t]
        # chunks are stored back-to-back as [p, chunk, b] blocks so each
        # tile is one contiguous DMA.
        xc = x[c * BC:(c + 1) * BC, 0].astype(bf16)                # (BC, 256, 8, 8)
        xg = xc.reshape(BC, 2, 128, 64).transpose(1, 3, 2, 0)      # (sblk, hw, s_in, b)
        xg = xg.reshape(G, 128, BC)                                # (g, p, b)
        parts, off = [], 0
        for n in XTILES:
            parts.append(np.ascontiguousarray(
                xg[off:off + n].transpose(1, 0, 2)).reshape(-1))   # (p, c, b) flat
            off += n
        in_maps.append({
            "xt": np.concatenate(parts), "wt": wt, "bi": bi,
        })
    return in_maps


def kernel(x, W, b, _trace=False):
    nc = _get_kernel()
    in_maps = make_in_maps(np.asarray(x), np.asarray(W), np.asarray(b))
    res = run_bass_kernel_spmd(nc, in_maps, list(range(NCORES)), trace=_trace)
    out = np.empty((B, F), np.float32)
    for c in range(NCORES):
        out[c * BC:(c + 1) * BC] = res.results[c]["outT"].T
    if _trace:
        return out, res
    return out
